# revision 2
# baseline (speedup 1.0000x reference)
"""Block-circulant matmul kernel for 8 Trainium2 NeuronCores.

Reference op (per token row x of shape (4096,)):
    y = (x*d) @ M + bias,  M[(j,m),(i,n)] = W[i,j,(m-n)%256]  (circulant blocks)

Implementation: radix-2 polyphase turns the 16x16 grid of 256-circulants into
a 32x32 grid of 128-circulants.  Per core (1024 tokens, data-parallel):
  stage1: per 128-block j2, one 128x128 real-DFT matmul (shared weights, bf16)
  stage2: per slot-pair q (2 DFT slots), one 128x128 frequency-mix matmul
  stage3: per output block i2, one 128x128 inverse-DFT matmul (shared weights)
Between stages, 4->128 partition-fanout SBUF->SBUF DMAs regroup the data
(block-major <-> slot-major).  All compute bf16 with fp32 PSUM accumulate;
HBM I/O is bf16 both ways.  Bias + final fp32 cast + layout gather on host.
"""
import os
import sys

for _p in ("/root/.axon_site", "/root/.axon_site/_ro/trn_rl_repo", "/root/.axon_site/_ro/pypackages"):
    if _p not in sys.path:
        sys.path.append(_p)

import numpy as np
import ml_dtypes

import concourse.bass as bass
import concourse.tile as tile
from concourse import bacc, mybir
from concourse import bass_utils

N_CORES = 8
B = 8192
D = 4096
BS = 256
K = 16             # 256-blocks per side
L = 128            # polyphase conv length
KB = 32            # 128-blocks per side (j2 = 2j + b)
NQ = 32            # slot-pairs; slot s = 32*sl + q
NT = B // N_CORES  # tokens per core (1024)
TC = 256           # token chunk
NCH = NT // TC     # chunks (4)

F32 = mybir.dt.float32
BF16 = mybir.dt.bfloat16
BF16_NP = ml_dtypes.bfloat16

LAST_EXEC_NS = None
_CACHE = {}


# ---------------------------------------------------------------- host math

def _host_mats(W):
    """T (stage-1 lhsT), MIX (stage-2 lhsT per q), R (stage-3 lhsT)."""
    s_idx = np.arange(L)
    W2 = np.empty((KB, KB, L), np.float64)
    for i2 in range(KB):
        i, a = i2 // 2, i2 % 2
        for j2 in range(KB):
            j, b = j2 // 2, j2 % 2
            W2[i2, j2] = W[i, j][(2 * s_idx + b - a) % BS]
    G = np.fft.fft(W2, axis=-1)
    Gr, Gi = G.real, G.imag

    v_idx = np.arange(L)
    T = np.zeros((L, L), np.float64)
    for q in range(NQ):
        for sl in range(2):
            s = 32 * sl + q
            for c in range(2):
                p = q * 4 + sl * 2 + c
                if s == 0:
                    T[:, p] = 1.0 if c == 0 else (-1.0) ** v_idx
                else:
                    T[:, p] = (np.cos(2 * np.pi * s * v_idx / L) if c == 0
                               else np.sin(2 * np.pi * s * v_idx / L))

    MIX = np.zeros((NQ, 128, 128), np.float64)
    kk = np.arange(KB)
    for q in range(NQ):
        for sl in range(2):
            s = 32 * sl + q
            for i2 in range(KB):
                for cp in range(2):
                    col = i2 * 4 + sl * 2 + cp
                    if s == 0:
                        f = 0 if cp == 0 else 64
                        MIX[q, kk + (sl * 2 + cp) * 32, col] = Gr[i2, :, f]
                    elif cp == 0:
                        MIX[q, kk + (sl * 2 + 0) * 32, col] = Gr[i2, :, s]
                        MIX[q, kk + (sl * 2 + 1) * 32, col] = -Gi[i2, :, s]
                    else:
                        MIX[q, kk + (sl * 2 + 0) * 32, col] = Gi[i2, :, s]
                        MIX[q, kk + (sl * 2 + 1) * 32, col] = Gr[i2, :, s]

    n_idx = np.arange(L)
    R = np.zeros((L, L), np.float64)
    for q in range(NQ):
        for sl in range(2):
            s = 32 * sl + q
            for cp in range(2):
                row = (sl * 2 + cp) * 32 + q
                if s == 0:
                    R[row, :] = 1.0 / L if cp == 0 else ((-1.0) ** n_idx) / L
                elif cp == 0:
                    R[row, :] = (2.0 / L) * np.cos(2 * np.pi * s * n_idx / L)
                else:
                    R[row, :] = (2.0 / L) * np.sin(2 * np.pi * s * n_idx / L)

    mix_flat = MIX.transpose(1, 0, 2).reshape(128, NQ * 128)  # [row, q*128+col]
    return (T.astype(BF16_NP), mix_flat.astype(BF16_NP), R.astype(BF16_NP))


# ---------------------------------------------------------------- device

def _build_nc():
    nc = bacc.Bacc("TRN2", target_bir_lowering=False, debug=False)
    x2 = nc.dram_tensor("x2", [128, NCH * KB * TC], BF16, kind="ExternalInput").ap()
    t_d = nc.dram_tensor("tmat", [128, 128], BF16, kind="ExternalInput").ap()
    mix_d = nc.dram_tensor("mix", [128, NQ * 128], BF16, kind="ExternalInput").ap()
    r_d = nc.dram_tensor("rmat", [128, 128], BF16, kind="ExternalInput").ap()
    y2 = nc.dram_tensor("y2", [128, NCH * KB * TC], BF16, kind="ExternalOutput").ap()

    # evac router: balance ACT vs DVE by modeled cost
    bal = [0.0, 0.0]

    def evac(dst, src, fd):
        act_c = (172 + fd) / 1.2
        dve_c = (120 + fd) / 0.96
        if bal[0] + act_c <= bal[1] + dve_c:
            bal[0] += act_c
            nc.scalar.copy(dst, src)
        else:
            bal[1] += dve_c
            nc.vector.tensor_copy(dst, src)

    with tile.TileContext(nc) as tc:
        with (
            tc.tile_pool(name="consts", bufs=1) as consts,
            tc.tile_pool(name="xpool", bufs=2) as xpool,
            tc.tile_pool(name="upool", bufs=2) as upool,
            tc.tile_pool(name="u2pool", bufs=2) as u2pool,
            tc.tile_pool(name="vpool", bufs=2) as vpool,
            tc.tile_pool(name="v2pool", bufs=2) as v2pool,
            tc.tile_pool(name="ypool", bufs=2) as ypool,
            tc.tile_pool(name="ps", bufs=2, space="PSUM") as pspool,
        ):
            t_sb = consts.tile([128, 128], BF16)
            nc.sync.dma_start(t_sb[:], t_d[:])
            mix_sb = consts.tile([128, NQ * 128], BF16)
            nc.sync.dma_start(mix_sb[:], mix_d[:])
            r_sb = consts.tile([128, 128], BF16)
            nc.sync.dma_start(r_sb[:], r_d[:])

            CW = KB * TC  # 8192 cols per chunk tile

            for ch in range(NCH):
                x_t = xpool.tile([128, CW], BF16, tag="x")
                nc.sync.dma_start(x_t[:], x2[:, ch * CW:(ch + 1) * CW])

                # ---- stage 1: per block j2, shared T weights ----
                u_t = upool.tile([128, CW], BF16, tag="u")
                for pg in range(4):
                    ps = pspool.tile([128, 2048], F32, tag="ps")
                    for k in range(8):
                        j2 = pg * 8 + k
                        nc.tensor.matmul(
                            ps[:, k * TC:(k + 1) * TC],
                            t_sb[:],
                            x_t[:, j2 * TC:(j2 + 1) * TC],
                            start=True, stop=True,
                        )
                    evac(u_t[:, pg * 2048:(pg + 1) * 2048], ps[:], 2048)

                # ---- shuffle1: u[q*4+g, (j2,t)] -> u2[(g,j2), (q,t)] ----
                u2_t = u2pool.tile([128, CW], BF16, tag="u2")
                for q in range(NQ):
                    nc.sync.dma_start(
                        u2_t[:, q * TC:(q + 1) * TC],
                        u_t[4 * q:4 * q + 4, :],
                    )

                # ---- stage 2: per slot-pair q ----
                v_t = vpool.tile([128, CW], BF16, tag="v")
                for qg in range(4):
                    ps = pspool.tile([128, 2048], F32, tag="ps")
                    for k in range(8):
                        q = qg * 8 + k
                        nc.tensor.matmul(
                            ps[:, k * TC:(k + 1) * TC],
                            mix_sb[:, q * 128:(q + 1) * 128],
                            u2_t[:, q * TC:(q + 1) * TC],
                            start=True, stop=True,
                        )
                    evac(v_t[:, qg * 2048:(qg + 1) * 2048], ps[:], 2048)

                # ---- shuffle2: v[i2*4+g', (q,t)] -> v2[(g',q), (i2,t)] ----
                v2_t = v2pool.tile([128, CW], BF16, tag="v2")
                for i2 in range(KB):
                    nc.scalar.dma_start(
                        v2_t[:, i2 * TC:(i2 + 1) * TC],
                        v_t[4 * i2:4 * i2 + 4, :],
                    )

                # ---- stage 3: per output block i2, shared R weights ----
                y_t = ypool.tile([128, CW], BF16, tag="y")
                for ig in range(4):
                    ps = pspool.tile([128, 2048], F32, tag="ps")
                    for k in range(8):
                        i2 = ig * 8 + k
                        nc.tensor.matmul(
                            ps[:, k * TC:(k + 1) * TC],
                            r_sb[:],
                            v2_t[:, i2 * TC:(i2 + 1) * TC],
                            start=True, stop=True,
                        )
                    evac(y_t[:, ig * 2048:(ig + 1) * 2048], ps[:], 2048)

                nc.scalar.dma_start(y2[:, ch * CW:(ch + 1) * CW], y_t[:])
    nc.compile()
    return nc


# ---------------------------------------------------------------- entry point

def _run(nc, in_maps):
    global LAST_EXEC_NS
    trace = bool(os.environ.get("BASS_TRACE"))
    res = bass_utils.run_bass_kernel_spmd(
        nc, in_maps, list(range(N_CORES)), trace=trace,
        tmpdir=os.environ.get("BASS_TRACE_DIR") or None,
    )
    LAST_EXEC_NS = res.exec_time_ns
    return res


def kernel(x, W, d_bernoulli, bias):
    x = np.asarray(x, dtype=np.float32)
    W = np.asarray(W, dtype=np.float32)
    d_bernoulli = np.asarray(d_bernoulli, dtype=np.float32)
    bias = np.asarray(bias, dtype=np.float32)

    if "nc" not in _CACHE:
        _CACHE["nc"] = _build_nc()
    tmat, mix, rmat = _host_mats(W.astype(np.float64))

    xd = (x * d_bernoulli[None, :]).astype(BF16_NP)
    # X2[v, j2, tok]: col = j*256 + 2v + b
    X2 = np.ascontiguousarray(
        xd.reshape(B, K, L, 2).transpose(2, 1, 3, 0).reshape(L, KB, B))

    in_maps = []
    for c in range(N_CORES):
        xc = X2[:, :, c * NT:(c + 1) * NT]               # [128, 32, 1024]
        xc = (xc.reshape(L, KB, NCH, TC).transpose(0, 2, 1, 3)
              .reshape(L, NCH * KB * TC))
        in_maps.append({
            "x2": np.ascontiguousarray(xc),
            "tmat": tmat, "mix": mix, "rmat": rmat,
        })
    res = _run(_CACHE["nc"], in_maps)

    out = np.empty((B, D), dtype=np.float32)
    for c in range(N_CORES):
        yd = res.results[c]["y2"]                        # [128, NCH*KB*TC] bf16
        yc = (yd.reshape(L, NCH, K, 2, TC).transpose(1, 4, 2, 0, 3)
              .reshape(NT, D).astype(np.float32))
        out[c * NT:(c + 1) * NT, :] = yc
    out += bias[None, :]
    return out


# revision 6
# speedup vs baseline: 1.1710x; 1.1710x over previous
"""Block-circulant matmul kernel for 8 Trainium2 NeuronCores.

Reference op (per token row x of shape (4096,)):
    y = (x*d) @ M + bias,  M[(j,m),(i,n)] = W[i,j,(m-n)%256]  (circulant blocks)

Implementation: radix-2 polyphase turns the 16x16 grid of 256-circulants into
a 32x32 grid of 128-circulants.  Per core (1024 tokens, data-parallel):
  stage1: per 128-block j2, one 128x128 real-DFT matmul (shared weights, bf16)
  stage2: per slot-pair q (2 DFT slots), one 128x128 frequency-mix matmul
  stage3: per output block i2, one 128x128 inverse-DFT matmul (shared weights)
Between stages, 4->128 partition-fanout SBUF->SBUF DMAs regroup the data
(block-major <-> slot-major).  All compute bf16 with fp32 PSUM accumulate;
HBM I/O is bf16 both ways.  Bias + final fp32 cast + layout gather on host.
"""
import os
import sys

for _p in ("/root/.axon_site", "/root/.axon_site/_ro/trn_rl_repo", "/root/.axon_site/_ro/pypackages"):
    if _p not in sys.path:
        sys.path.append(_p)

import numpy as np
import ml_dtypes

import concourse.bass as bass
import concourse.tile as tile
from concourse import bacc, mybir
from concourse import bass_utils

N_CORES = 8
B = 8192
D = 4096
BS = 256
K = 16             # 256-blocks per side
L = 128            # polyphase conv length
KB = 32            # 128-blocks per side (j2 = 2j + b)
NQ = 32            # slot-pairs; slot s = 32*sl + q
NT = B // N_CORES  # tokens per core (1024)
TC = 256           # token chunk
NCH = NT // TC     # chunks (4)

F32 = mybir.dt.float32
BF16 = mybir.dt.bfloat16
BF16_NP = ml_dtypes.bfloat16

LAST_EXEC_NS = None
_CACHE = {}


# ---------------------------------------------------------------- host math

def _host_mats(W):
    """T (stage-1 lhsT), MIX (stage-2 lhsT per q), R (stage-3 lhsT)."""
    s_idx = np.arange(L)
    W2 = np.empty((KB, KB, L), np.float64)
    for i2 in range(KB):
        i, a = i2 // 2, i2 % 2
        for j2 in range(KB):
            j, b = j2 // 2, j2 % 2
            W2[i2, j2] = W[i, j][(2 * s_idx + b - a) % BS]
    G = np.fft.fft(W2, axis=-1)
    Gr, Gi = G.real, G.imag

    v_idx = np.arange(L)
    T = np.zeros((L, L), np.float64)
    for q in range(NQ):
        for sl in range(2):
            s = 32 * sl + q
            for c in range(2):
                p = (sl * 2 + c) * 32 + q
                if s == 0:
                    T[:, p] = 1.0 if c == 0 else (-1.0) ** v_idx
                else:
                    T[:, p] = (np.cos(2 * np.pi * s * v_idx / L) if c == 0
                               else np.sin(2 * np.pi * s * v_idx / L))

    MIX = np.zeros((NQ, 128, 128), np.float64)
    kk = np.arange(KB)
    for q in range(NQ):
        for sl in range(2):
            s = 32 * sl + q
            for i2 in range(KB):
                for cp in range(2):
                    col = (sl * 2 + cp) * 32 + i2
                    if s == 0:
                        f = 0 if cp == 0 else 64
                        MIX[q, kk + (sl * 2 + cp) * 32, col] = Gr[i2, :, f]
                    elif cp == 0:
                        MIX[q, kk + (sl * 2 + 0) * 32, col] = Gr[i2, :, s]
                        MIX[q, kk + (sl * 2 + 1) * 32, col] = -Gi[i2, :, s]
                    else:
                        MIX[q, kk + (sl * 2 + 0) * 32, col] = Gi[i2, :, s]
                        MIX[q, kk + (sl * 2 + 1) * 32, col] = Gr[i2, :, s]

    n_idx = np.arange(L)
    R = np.zeros((L, L), np.float64)
    for q in range(NQ):
        for sl in range(2):
            s = 32 * sl + q
            for cp in range(2):
                row = (sl * 2 + cp) * 32 + q
                if s == 0:
                    R[row, :] = 1.0 / L if cp == 0 else ((-1.0) ** n_idx) / L
                elif cp == 0:
                    R[row, :] = (2.0 / L) * np.cos(2 * np.pi * s * n_idx / L)
                else:
                    R[row, :] = (2.0 / L) * np.sin(2 * np.pi * s * n_idx / L)

    mix_flat = MIX.transpose(1, 0, 2).reshape(128, NQ * 128)  # [row, q*128+col]
    return (T.astype(BF16_NP), mix_flat.astype(BF16_NP), R.astype(BF16_NP))


# ---------------------------------------------------------------- device

def _build_nc():
    nc = bacc.Bacc("TRN2", target_bir_lowering=False, debug=False)
    x2 = nc.dram_tensor("x2", [128, NCH * KB * TC], BF16, kind="ExternalInput").ap()
    t_d = nc.dram_tensor("tmat", [128, 128], BF16, kind="ExternalInput").ap()
    mix_d = nc.dram_tensor("mix", [128, NQ * 128], BF16, kind="ExternalInput").ap()
    r_d = nc.dram_tensor("rmat", [128, 128], BF16, kind="ExternalInput").ap()
    y2 = nc.dram_tensor("y2", [128, NCH * KB * TC], BF16, kind="ExternalOutput").ap()

    # evac router: balance ACT vs DVE by modeled cost (DVE also does the
    # shuffle transposes, accounted via bal[1])
    bal = [0.0, 0.0]

    def evac(dst, src, fd):
        act_c = (172 + fd) / 1.2
        dve_c = (120 + fd) / 0.96
        if bal[0] + act_c <= bal[1] + dve_c:
            bal[0] += act_c
            nc.scalar.copy(dst, src)
        else:
            bal[1] += dve_c
            nc.vector.tensor_copy(dst, src)

    def shuffle(dst_tj, src_tj, fd):
        # 32x32 block transpose per partition-group: swaps the within-group
        # partition index with the inner free index (both APs [128, t, 32])
        bal[1] += (58 + fd) / 0.96
        nc.vector.transpose(dst_tj, src_tj)

    with tile.TileContext(nc) as tc:
        with (
            tc.tile_pool(name="consts", bufs=1) as consts,
            tc.tile_pool(name="xpool", bufs=2) as xpool,
            tc.tile_pool(name="upool", bufs=2) as upool,
            tc.tile_pool(name="u2pool", bufs=2) as u2pool,
            tc.tile_pool(name="vpool", bufs=2) as vpool,
            tc.tile_pool(name="v2pool", bufs=2) as v2pool,
            tc.tile_pool(name="ypool", bufs=2) as ypool,
            tc.tile_pool(name="ps", bufs=2, space="PSUM") as pspool,
        ):
            t_sb = consts.tile([128, 128], BF16)
            nc.sync.dma_start(t_sb[:], t_d[:])
            mix_sb = consts.tile([128, NQ * 128], BF16)
            nc.sync.dma_start(mix_sb[:], mix_d[:])
            r_sb = consts.tile([128, 128], BF16)
            nc.sync.dma_start(r_sb[:], r_d[:])

            CW = KB * TC  # 8192 cols per chunk tile

            for ch in range(NCH):
                x_t = xpool.tile([128, CW], BF16, tag="x")
                nc.sync.dma_start(x_t[:], x2[:, ch * CW:(ch + 1) * CW])

                # ---- stage 1: per block j2, shared T weights ----
                u_t = upool.tile([128, CW], BF16, tag="u")
                for pg in range(4):
                    ps = pspool.tile([128, 2048], F32, tag="ps")
                    for k in range(8):
                        j2 = pg * 8 + k
                        nc.tensor.matmul(
                            ps[:, k * TC:(k + 1) * TC],
                            t_sb[:],
                            x_t[:, j2 * TC:(j2 + 1) * TC],
                            start=True, stop=True,
                        )
                    evac(u_t[:, pg * 2048:(pg + 1) * 2048], ps[:], 2048)

                # ---- shuffle1: u[(g,q), (j2,t)] -> u2[(g,j2), (q,t)] ----
                u2_t = u2pool.tile([128, CW], BF16, tag="u2")
                shuffle(
                    u2_t[:].rearrange("p (q t) -> p t q", q=NQ),
                    u_t[:].rearrange("p (j t) -> p t j", j=KB),
                    CW,
                )

                # ---- stage 2: per slot-pair q ----
                v_t = vpool.tile([128, CW], BF16, tag="v")
                for qg in range(4):
                    ps = pspool.tile([128, 2048], F32, tag="ps")
                    for k in range(8):
                        q = qg * 8 + k
                        nc.tensor.matmul(
                            ps[:, k * TC:(k + 1) * TC],
                            mix_sb[:, q * 128:(q + 1) * 128],
                            u2_t[:, q * TC:(q + 1) * TC],
                            start=True, stop=True,
                        )
                    evac(v_t[:, qg * 2048:(qg + 1) * 2048], ps[:], 2048)

                # ---- shuffle2: v[(g',i2), (q,t)] -> v2[(g',q), (i2,t)] ----
                v2_t = v2pool.tile([128, CW], BF16, tag="v2")
                shuffle(
                    v2_t[:].rearrange("p (i t) -> p t i", i=KB),
                    v_t[:].rearrange("p (q t) -> p t q", q=NQ),
                    CW,
                )

                # ---- stage 3: per output block i2, shared R weights ----
                y_t = ypool.tile([128, CW], BF16, tag="y")
                for ig in range(4):
                    ps = pspool.tile([128, 2048], F32, tag="ps")
                    for k in range(8):
                        i2 = ig * 8 + k
                        nc.tensor.matmul(
                            ps[:, k * TC:(k + 1) * TC],
                            r_sb[:],
                            v2_t[:, i2 * TC:(i2 + 1) * TC],
                            start=True, stop=True,
                        )
                    evac(y_t[:, ig * 2048:(ig + 1) * 2048], ps[:], 2048)

                nc.scalar.dma_start(y2[:, ch * CW:(ch + 1) * CW], y_t[:])
    nc.compile()
    return nc


# ---------------------------------------------------------------- entry point

def _run(nc, in_maps):
    global LAST_EXEC_NS
    trace = bool(os.environ.get("BASS_TRACE"))
    res = bass_utils.run_bass_kernel_spmd(
        nc, in_maps, list(range(N_CORES)), trace=trace,
        tmpdir=os.environ.get("BASS_TRACE_DIR") or None,
    )
    LAST_EXEC_NS = res.exec_time_ns
    return res


def kernel(x, W, d_bernoulli, bias):
    x = np.asarray(x, dtype=np.float32)
    W = np.asarray(W, dtype=np.float32)
    d_bernoulli = np.asarray(d_bernoulli, dtype=np.float32)
    bias = np.asarray(bias, dtype=np.float32)

    if "nc" not in _CACHE:
        _CACHE["nc"] = _build_nc()
    tmat, mix, rmat = _host_mats(W.astype(np.float64))

    xd = (x * d_bernoulli[None, :]).astype(BF16_NP)
    # X2[v, j2, tok]: col = j*256 + 2v + b
    X2 = np.ascontiguousarray(
        xd.reshape(B, K, L, 2).transpose(2, 1, 3, 0).reshape(L, KB, B))

    in_maps = []
    for c in range(N_CORES):
        xc = X2[:, :, c * NT:(c + 1) * NT]               # [128, 32, 1024]
        xc = (xc.reshape(L, KB, NCH, TC).transpose(0, 2, 1, 3)
              .reshape(L, NCH * KB * TC))
        in_maps.append({
            "x2": np.ascontiguousarray(xc),
            "tmat": tmat, "mix": mix, "rmat": rmat,
        })
    res = _run(_CACHE["nc"], in_maps)

    out = np.empty((B, D), dtype=np.float32)
    for c in range(N_CORES):
        yd = res.results[c]["y2"]                        # [128, NCH*KB*TC] bf16
        yc = (yd.reshape(L, NCH, K, 2, TC).transpose(1, 4, 2, 0, 3)
              .reshape(NT, D).astype(np.float32))
        out[c * NT:(c + 1) * NT, :] = yc
    out += bias[None, :]
    return out


# revision 8
# speedup vs baseline: 1.4411x; 1.2306x over previous
"""Block-circulant matmul kernel for 8 Trainium2 NeuronCores.

Reference op (per token row x of shape (4096,)):
    y = (x*d) @ M + bias,  M[(j,m),(i,n)] = W[i,j,(m-n)%256]  (circulant blocks)

Implementation: radix-2 polyphase turns the 16x16 grid of 256-circulants into
a 32x32 grid of 128-circulants.  Per core (1024 tokens, data-parallel):
  stage1: per 128-block j2, one 128x128 real-DFT matmul (shared weights, bf16)
  stage2: per slot-pair q (2 DFT slots), one 128x128 frequency-mix matmul
  stage3: per output block i2, one 128x128 inverse-DFT matmul (shared weights)
Between stages, 4->128 partition-fanout SBUF->SBUF DMAs regroup the data
(block-major <-> slot-major).  All compute bf16 with fp32 PSUM accumulate;
HBM I/O is bf16 both ways.  Bias + final fp32 cast + layout gather on host.
"""
import os
import sys

for _p in ("/root/.axon_site", "/root/.axon_site/_ro/trn_rl_repo", "/root/.axon_site/_ro/pypackages"):
    if _p not in sys.path:
        sys.path.append(_p)

import numpy as np
import ml_dtypes

import concourse.bass as bass
import concourse.tile as tile
from concourse import bacc, mybir
from concourse import bass_utils

N_CORES = 8
B = 8192
D = 4096
BS = 256
K = 16             # 256-blocks per side
L = 128            # polyphase conv length
KB = 32            # 128-blocks per side (j2 = 2j + b)
NQ = 32            # slot-pairs; slot s = 32*sl + q
NT = B // N_CORES  # tokens per core (1024)
TC = 256           # token chunk
NCH = NT // TC     # chunks (4)

F32 = mybir.dt.float32
BF16 = mybir.dt.bfloat16
BF16_NP = ml_dtypes.bfloat16

LAST_EXEC_NS = None
_CACHE = {}


# ---------------------------------------------------------------- host math

def _host_mats(W):
    """T (stage-1 lhsT), MIX (stage-2 lhsT per q), R (stage-3 lhsT)."""
    s_idx = np.arange(L)
    W2 = np.empty((KB, KB, L), np.float64)
    for i2 in range(KB):
        i, a = i2 // 2, i2 % 2
        for j2 in range(KB):
            j, b = j2 // 2, j2 % 2
            W2[i2, j2] = W[i, j][(2 * s_idx + b - a) % BS]
    G = np.fft.fft(W2, axis=-1)
    Gr, Gi = G.real, G.imag

    v_idx = np.arange(L)
    T = np.zeros((L, L), np.float64)
    for q in range(NQ):
        for sl in range(2):
            s = 32 * sl + q
            for c in range(2):
                p = (sl * 2 + c) * 32 + q
                if s == 0:
                    T[:, p] = 1.0 if c == 0 else (-1.0) ** v_idx
                else:
                    T[:, p] = (np.cos(2 * np.pi * s * v_idx / L) if c == 0
                               else np.sin(2 * np.pi * s * v_idx / L))

    MIX = np.zeros((NQ, 128, 128), np.float64)
    kk = np.arange(KB)
    for q in range(NQ):
        for sl in range(2):
            s = 32 * sl + q
            for i2 in range(KB):
                for cp in range(2):
                    col = i2 * 4 + sl * 2 + cp
                    if s == 0:
                        f = 0 if cp == 0 else 64
                        MIX[q, kk + (sl * 2 + cp) * 32, col] = Gr[i2, :, f]
                    elif cp == 0:
                        MIX[q, kk + (sl * 2 + 0) * 32, col] = Gr[i2, :, s]
                        MIX[q, kk + (sl * 2 + 1) * 32, col] = -Gi[i2, :, s]
                    else:
                        MIX[q, kk + (sl * 2 + 0) * 32, col] = Gi[i2, :, s]
                        MIX[q, kk + (sl * 2 + 1) * 32, col] = Gr[i2, :, s]

    n_idx = np.arange(L)
    R = np.zeros((L, L), np.float64)
    for q in range(NQ):
        for sl in range(2):
            s = 32 * sl + q
            for cp in range(2):
                row = (sl * 2 + cp) * 32 + q
                if s == 0:
                    R[row, :] = 1.0 / L if cp == 0 else ((-1.0) ** n_idx) / L
                elif cp == 0:
                    R[row, :] = (2.0 / L) * np.cos(2 * np.pi * s * n_idx / L)
                else:
                    R[row, :] = (2.0 / L) * np.sin(2 * np.pi * s * n_idx / L)

    mix_flat = MIX.transpose(1, 0, 2).reshape(128, NQ * 128)  # [row, q*128+col]
    return (T.astype(BF16_NP), mix_flat.astype(BF16_NP), R.astype(BF16_NP))


# ---------------------------------------------------------------- device

def _build_nc():
    nc = bacc.Bacc("TRN2", target_bir_lowering=False, debug=False)
    x2 = nc.dram_tensor("x2", [128, NCH * KB * TC], BF16, kind="ExternalInput").ap()
    t_d = nc.dram_tensor("tmat", [128, 128], BF16, kind="ExternalInput").ap()
    mix_d = nc.dram_tensor("mix", [128, NQ * 128], BF16, kind="ExternalInput").ap()
    r_d = nc.dram_tensor("rmat", [128, 128], BF16, kind="ExternalInput").ap()
    y2 = nc.dram_tensor("y2", [128, NCH * KB * TC], BF16, kind="ExternalOutput").ap()

    # evac router: balance ACT vs DVE by modeled cost (DVE also does the
    # shuffle-1 transposes, accounted via bal[1])
    bal = [0.0, 0.0]

    def evac(dst, src, fd):
        act_c = (172 + fd) / 1.2
        dve_c = (120 + fd) / 0.96
        if bal[0] + act_c <= bal[1] + dve_c:
            bal[0] += act_c
            nc.scalar.copy(dst, src)
        else:
            bal[1] += dve_c
            nc.vector.tensor_copy(dst, src)

    TH = 512           # shuffle-2 DMA token granularity (half = 2 chunks)
    CW = KB * TC       # 8192 cols per chunk tile

    with tile.TileContext(nc) as tc:
        with (
            tc.tile_pool(name="consts", bufs=1) as consts,
            tc.tile_pool(name="xpool", bufs=2) as xpool,
            tc.tile_pool(name="upool", bufs=2) as upool,
            tc.tile_pool(name="u2pool", bufs=1) as u2pool,
            tc.tile_pool(name="vpool", bufs=2) as vpool,
            tc.tile_pool(name="v2pool", bufs=1) as v2pool,
            tc.tile_pool(name="ypool", bufs=1) as ypool,
            tc.tile_pool(name="ps", bufs=2, space="PSUM") as pspool,
        ):
            t_sb = consts.tile([128, 128], BF16)
            nc.sync.dma_start(t_sb[:], t_d[:])
            mix_sb = consts.tile([128, NQ * 128], BF16)
            nc.sync.dma_start(mix_sb[:], mix_d[:])
            r_sb = consts.tile([128, 128], BF16)
            nc.sync.dma_start(r_sb[:], r_d[:])

            for h in range(NT // TH):
                # v: [128 = (i2,g'), (q, t_TH)]  (q-major runs for shuffle-2)
                v_t = vpool.tile([128, NQ * TH], BF16, tag="v")
                for cc in range(TH // TC):
                    ch = h * (TH // TC) + cc
                    x_t = xpool.tile([128, CW], BF16, tag="x")
                    nc.sync.dma_start(x_t[:], x2[:, ch * CW:(ch + 1) * CW])

                    # ---- stage 1: per block j2, shared T weights ----
                    # u: [128 = (g,q), (t, j2)]  (t-major for the transpose)
                    u_t = upool.tile([128, CW], BF16, tag="u")
                    u_jt = u_t[:].rearrange("p (t j) -> p j t", j=KB)
                    for pg in range(4):
                        ps = pspool.tile([128, 2048], F32, tag="ps")
                        for k in range(8):
                            j2 = pg * 8 + k
                            nc.tensor.matmul(
                                ps[:, k * TC:(k + 1) * TC],
                                t_sb[:],
                                x_t[:, j2 * TC:(j2 + 1) * TC],
                                start=True, stop=True,
                            )
                        evac(u_jt[:, pg * 8:(pg + 1) * 8, :], ps[:], 2048)

                    # ---- shuffle1 (DVE 32x32 block transpose, contiguous
                    # squares): u[(g,q),(t,j2)] -> u2[(g,j2),(t,q)] ----
                    u2_t = u2pool.tile([128, CW], BF16, tag="u2")
                    bal[1] += (58 + CW) / 0.96
                    nc.vector.transpose(
                        u2_t[:].rearrange("p (t q) -> p t q", q=NQ),
                        u_t[:].rearrange("p (t j) -> p t j", j=KB),
                    )

                    # ---- stage 2: per slot-pair q (strided moving rhs) ----
                    u2_qt = u2_t[:].rearrange("p (t q) -> p q t", q=NQ)
                    v_qt = v_t[:].rearrange("p (q t) -> p q t", q=NQ)
                    for qg in range(4):
                        ps = pspool.tile([128, 2048], F32, tag="ps")
                        for k in range(8):
                            q = qg * 8 + k
                            nc.tensor.matmul(
                                ps[:, k * TC:(k + 1) * TC],
                                mix_sb[:, q * 128:(q + 1) * 128],
                                u2_qt[:, q, :],
                                start=True, stop=True,
                            )
                        evac(v_qt[:, qg * 8:(qg + 1) * 8,
                                  cc * TC:(cc + 1) * TC], ps[:], 2048)

                # ---- shuffle2 (gpsimd SWDGE 4->128 fanout per i2):
                # v[i2*4+g', (q,t)] -> v2[(g',q), (i2,t)] ----
                v2_t = v2pool.tile([128, KB * TH], BF16, tag="v2")
                for i2 in range(KB):
                    nc.gpsimd.dma_start(
                        v2_t[:, i2 * TH:(i2 + 1) * TH],
                        v_t[4 * i2:4 * i2 + 4, :],
                    )

                # ---- stage 3: per output block i2, shared R weights ----
                for cc in range(TH // TC):
                    ch = h * (TH // TC) + cc
                    y_t = ypool.tile([128, CW], BF16, tag="y")
                    for ig in range(4):
                        ps = pspool.tile([128, 2048], F32, tag="ps")
                        for k in range(8):
                            i2 = ig * 8 + k
                            nc.tensor.matmul(
                                ps[:, k * TC:(k + 1) * TC],
                                r_sb[:],
                                v2_t[:, i2 * TH + cc * TC:
                                     i2 * TH + (cc + 1) * TC],
                                start=True, stop=True,
                            )
                        evac(y_t[:, ig * 2048:(ig + 1) * 2048], ps[:], 2048)
                    nc.scalar.dma_start(y2[:, ch * CW:(ch + 1) * CW], y_t[:])
    nc.compile()
    return nc


# ---------------------------------------------------------------- entry point

def _run(nc, in_maps):
    global LAST_EXEC_NS
    trace = bool(os.environ.get("BASS_TRACE"))
    res = bass_utils.run_bass_kernel_spmd(
        nc, in_maps, list(range(N_CORES)), trace=trace,
        tmpdir=os.environ.get("BASS_TRACE_DIR") or None,
    )
    LAST_EXEC_NS = res.exec_time_ns
    return res


def kernel(x, W, d_bernoulli, bias):
    x = np.asarray(x, dtype=np.float32)
    W = np.asarray(W, dtype=np.float32)
    d_bernoulli = np.asarray(d_bernoulli, dtype=np.float32)
    bias = np.asarray(bias, dtype=np.float32)

    if "nc" not in _CACHE:
        _CACHE["nc"] = _build_nc()
    tmat, mix, rmat = _host_mats(W.astype(np.float64))

    xd = (x * d_bernoulli[None, :]).astype(BF16_NP)
    # X2[v, j2, tok]: col = j*256 + 2v + b
    X2 = np.ascontiguousarray(
        xd.reshape(B, K, L, 2).transpose(2, 1, 3, 0).reshape(L, KB, B))

    in_maps = []
    for c in range(N_CORES):
        xc = X2[:, :, c * NT:(c + 1) * NT]               # [128, 32, 1024]
        xc = (xc.reshape(L, KB, NCH, TC).transpose(0, 2, 1, 3)
              .reshape(L, NCH * KB * TC))
        in_maps.append({
            "x2": np.ascontiguousarray(xc),
            "tmat": tmat, "mix": mix, "rmat": rmat,
        })
    res = _run(_CACHE["nc"], in_maps)

    out = np.empty((B, D), dtype=np.float32)
    for c in range(N_CORES):
        yd = res.results[c]["y2"]                        # [128, NCH*KB*TC] bf16
        yc = (yd.reshape(L, NCH, K, 2, TC).transpose(1, 4, 2, 0, 3)
              .reshape(NT, D).astype(np.float32))
        out[c * NT:(c + 1) * NT, :] = yc
    out += bias[None, :]
    return out


# revision 14
# speedup vs baseline: 1.7452x; 1.2111x over previous
"""Block-circulant matmul kernel for 8 Trainium2 NeuronCores.

Reference op (per token row x of shape (4096,)):
    y = (x*d) @ M + bias,  M[(j,m),(i,n)] = W[i,j,(m-n)%256]  (circulant blocks)

Implementation: radix-2 polyphase turns the 16x16 grid of 256-circulants into
a 32x32 grid of 128-circulants.  Per core (1024 tokens, data-parallel):
  stage1: per 128-block j2, one 128x128 real-DFT matmul (shared weights, bf16)
  stage2: per slot-pair q (2 DFT slots), one 128x128 frequency-mix matmul
  stage3: per output block i2, one 128x128 inverse-DFT matmul (shared weights)
Between stages, 4->128 partition-fanout SBUF->SBUF DMAs regroup the data
(block-major <-> slot-major).  All compute bf16 with fp32 PSUM accumulate;
HBM I/O is bf16 both ways.  Bias + final fp32 cast + layout gather on host.
"""
import os
import sys

for _p in ("/root/.axon_site", "/root/.axon_site/_ro/trn_rl_repo", "/root/.axon_site/_ro/pypackages"):
    if _p not in sys.path:
        sys.path.append(_p)

import numpy as np
import ml_dtypes

import concourse.bass as bass
import concourse.tile as tile
from concourse import bacc, mybir
from concourse import bass_utils

N_CORES = 8
B = 8192
D = 4096
BS = 256
K = 16             # 256-blocks per side
L = 128            # polyphase conv length
KB = 32            # 128-blocks per side (j2 = 2j + b)
NQ = 32            # slot-pairs; slot s = 32*sl + q
NT = B // N_CORES  # tokens per core (1024)
TC = 256           # token chunk
NCH = NT // TC     # chunks (4)

F32 = mybir.dt.float32
BF16 = mybir.dt.bfloat16
BF16_NP = ml_dtypes.bfloat16

LAST_EXEC_NS = None
_CACHE = {}


# ---------------------------------------------------------------- host math

def _host_mats(W):
    """T (stage-1 lhsT), MIX (stage-2 lhsT per q), R (stage-3 lhsT)."""
    s_idx = np.arange(L)
    W2 = np.empty((KB, KB, L), np.float64)
    for i2 in range(KB):
        i, a = i2 // 2, i2 % 2
        for j2 in range(KB):
            j, b = j2 // 2, j2 % 2
            W2[i2, j2] = W[i, j][(2 * s_idx + b - a) % BS]
    G = np.fft.fft(W2, axis=-1)
    Gr, Gi = G.real, G.imag

    v_idx = np.arange(L)
    T = np.zeros((L, L), np.float64)
    for q in range(NQ):
        for sl in range(2):
            s = 32 * sl + q
            for c in range(2):
                p = q * 4 + sl * 2 + c
                if s == 0:
                    T[:, p] = 1.0 if c == 0 else (-1.0) ** v_idx
                else:
                    T[:, p] = (np.cos(2 * np.pi * s * v_idx / L) if c == 0
                               else np.sin(2 * np.pi * s * v_idx / L))

    MIX = np.zeros((NQ, 128, 128), np.float64)
    kk = np.arange(KB)
    for q in range(NQ):
        for sl in range(2):
            s = 32 * sl + q
            for i2 in range(KB):
                for cp in range(2):
                    col = i2 * 4 + sl * 2 + cp
                    if s == 0:
                        f = 0 if cp == 0 else 64
                        MIX[q, kk + (sl * 2 + cp) * 32, col] = Gr[i2, :, f]
                    elif cp == 0:
                        MIX[q, kk + (sl * 2 + 0) * 32, col] = Gr[i2, :, s]
                        MIX[q, kk + (sl * 2 + 1) * 32, col] = -Gi[i2, :, s]
                    else:
                        MIX[q, kk + (sl * 2 + 0) * 32, col] = Gi[i2, :, s]
                        MIX[q, kk + (sl * 2 + 1) * 32, col] = Gr[i2, :, s]

    n_idx = np.arange(L)
    R = np.zeros((L, L), np.float64)
    for q in range(NQ):
        for sl in range(2):
            s = 32 * sl + q
            for cp in range(2):
                row = (sl * 2 + cp) * 32 + q
                if s == 0:
                    R[row, :] = 1.0 / L if cp == 0 else ((-1.0) ** n_idx) / L
                elif cp == 0:
                    R[row, :] = (2.0 / L) * np.cos(2 * np.pi * s * n_idx / L)
                else:
                    R[row, :] = (2.0 / L) * np.sin(2 * np.pi * s * n_idx / L)

    mix_flat = MIX.transpose(1, 0, 2).reshape(128, NQ * 128)  # [row, q*128+col]
    return (T.astype(BF16_NP), mix_flat.astype(BF16_NP), R.astype(BF16_NP))


# ---------------------------------------------------------------- device

def _build_nc():
    nc = bacc.Bacc("TRN2", target_bir_lowering=False, debug=False)
    x2 = nc.dram_tensor("x2", [128, NCH * KB * TC], BF16, kind="ExternalInput").ap()
    t_d = nc.dram_tensor("tmat", [128, 128], BF16, kind="ExternalInput").ap()
    mix_d = nc.dram_tensor("mix", [128, NQ * 128], BF16, kind="ExternalInput").ap()
    r_d = nc.dram_tensor("rmat", [128, 128], BF16, kind="ExternalInput").ap()
    y2 = nc.dram_tensor("y2", [128, NCH * KB * TC], BF16, kind="ExternalOutput").ap()

    # evac router: balance ACT vs DVE by modeled cost (DVE also does the
    # shuffle-1 transposes, accounted via bal[1])
    bal = [0.0, 0.0]

    def evac(dst, src, fd):
        act_c = (172 + fd) / 1.2
        dve_c = (120 + fd) / 0.96
        if bal[0] + act_c <= bal[1] + dve_c:
            bal[0] += act_c
            nc.scalar.copy(dst, src)
        else:
            bal[1] += dve_c
            nc.vector.tensor_copy(dst, src)

    CW = KB * TC       # 8192 cols per chunk tile

    def shuf_dma(k, dst, src):
        # split fan-out DMAs between SWDGE (gpsimd, ~0.9us) and HWDGE
        # (sync, ~1.5us) queues, ~5:3
        if k % 8 in (3, 6):
            nc.sync.dma_start(dst, src)
        else:
            nc.gpsimd.dma_start(dst, src)

    with tile.TileContext(nc) as tc:
        with (
            tc.tile_pool(name="consts", bufs=1) as consts,
            tc.tile_pool(name="xpool", bufs=2) as xpool,
            tc.tile_pool(name="p1", bufs=1) as p1,      # u then v (64KB)
            tc.tile_pool(name="p2", bufs=1) as p2,      # u2 then v2 (64KB)
            tc.tile_pool(name="ypool", bufs=4) as ypool,
            tc.tile_pool(name="ps", bufs=2, space="PSUM") as pspool,
        ):
            t_sb = consts.tile([128, 128], BF16)
            nc.sync.dma_start(t_sb[:], t_d[:])
            mix_sb = consts.tile([128, NQ * 128], BF16)
            nc.sync.dma_start(mix_sb[:], mix_d[:])
            r_sb = consts.tile([128, 128], BF16)
            nc.sync.dma_start(r_sb[:], r_d[:])

            # ---- stage 1 (per 256-token chunk): u[q*4+g, (j2, t_NT)] ----
            u_t = p1.tile([128, KB * NT], BF16, tag="uv", name="u_t")
            u_jt = u_t[:].rearrange("p (j t) -> p j t", j=KB)
            for ch in range(NCH):
                x_t = xpool.tile([128, CW], BF16, tag="x")
                nc.sync.dma_start(x_t[:], x2[:, ch * CW:(ch + 1) * CW])
                for pg in range(4):
                    ps = pspool.tile([128, 2048], F32, tag="ps")
                    for k in range(8):
                        j2 = pg * 8 + k
                        nc.tensor.matmul(
                            ps[:, k * TC:(k + 1) * TC],
                            t_sb[:],
                            x_t[:, j2 * TC:(j2 + 1) * TC],
                            start=True, stop=True,
                        )
                    evac(u_jt[:, pg * 8:(pg + 1) * 8,
                              ch * TC:(ch + 1) * TC], ps[:], 2048)

            # ---- shuffle1 + stage 2, per slot-pair q ----
            # u2[(g,j2), (q, t_NT)] <- u[4q:4q+4, :]; v[(i2,g'), (q, t_NT)]
            u2_t = p2.tile([128, NQ * NT], BF16, tag="uv2", name="u2_t")
            for q in range(NQ):
                shuf_dma(q, u2_t[:, q * NT:(q + 1) * NT], u_t[4 * q:4 * q + 4, :])
            v_t = p1.tile([128, NQ * NT], BF16, tag="uv", name="v_t")
            for qg in range(16):
                ps = pspool.tile([128, 2048], F32, tag="ps")
                for k in range(4):
                    q, th = 2 * qg + k // 2, k % 2
                    nc.tensor.matmul(
                        ps[:, k * 512:(k + 1) * 512],
                        mix_sb[:, q * 128:(q + 1) * 128],
                        u2_t[:, q * NT + th * 512:q * NT + (th + 1) * 512],
                        start=True, stop=True,
                    )
                evac(v_t[:, qg * 2048:(qg + 1) * 2048], ps[:], 2048)

            # ---- shuffle2 + stage 3, per output block i2 ----
            v2_t = p2.tile([128, KB * NT], BF16, tag="uv2", name="v2_t")
            for i2 in range(KB):
                shuf_dma(i2, v2_t[:, i2 * NT:(i2 + 1) * NT],
                         v_t[4 * i2:4 * i2 + 4, :])
            for ig in range(16):
                ps = pspool.tile([128, 2048], F32, tag="ps")
                for k in range(4):
                    i2, th = 2 * ig + k // 2, k % 2
                    nc.tensor.matmul(
                        ps[:, k * 512:(k + 1) * 512],
                        r_sb[:],
                        v2_t[:, i2 * NT + th * 512:i2 * NT + (th + 1) * 512],
                        start=True, stop=True,
                    )
                y_t = ypool.tile([128, 2048], BF16, tag="y")
                evac(y_t[:], ps[:], 2048)
                nc.sync.dma_start(y2[:, ig * 2048:(ig + 1) * 2048], y_t[:])
    nc.compile()
    return nc


# ---------------------------------------------------------------- entry point

def _run(nc, in_maps):
    global LAST_EXEC_NS
    trace = bool(os.environ.get("BASS_TRACE"))
    res = bass_utils.run_bass_kernel_spmd(
        nc, in_maps, list(range(N_CORES)), trace=trace,
        tmpdir=os.environ.get("BASS_TRACE_DIR") or None,
    )
    LAST_EXEC_NS = res.exec_time_ns
    return res


def kernel(x, W, d_bernoulli, bias):
    x = np.asarray(x, dtype=np.float32)
    W = np.asarray(W, dtype=np.float32)
    d_bernoulli = np.asarray(d_bernoulli, dtype=np.float32)
    bias = np.asarray(bias, dtype=np.float32)

    if "nc" not in _CACHE:
        _CACHE["nc"] = _build_nc()
    tmat, mix, rmat = _host_mats(W.astype(np.float64))

    xd = (x * d_bernoulli[None, :]).astype(BF16_NP)
    # X2[v, j2, tok]: col = j*256 + 2v + b
    X2 = np.ascontiguousarray(
        xd.reshape(B, K, L, 2).transpose(2, 1, 3, 0).reshape(L, KB, B))

    in_maps = []
    for c in range(N_CORES):
        xc = X2[:, :, c * NT:(c + 1) * NT]               # [128, 32, 1024]
        xc = (xc.reshape(L, KB, NCH, TC).transpose(0, 2, 1, 3)
              .reshape(L, NCH * KB * TC))
        in_maps.append({
            "x2": np.ascontiguousarray(xc),
            "tmat": tmat, "mix": mix, "rmat": rmat,
        })
    res = _run(_CACHE["nc"], in_maps)

    out = np.empty((B, D), dtype=np.float32)
    for c in range(N_CORES):
        yd = res.results[c]["y2"]                        # [128, (i2, t)] bf16
        yc = (yd.reshape(L, K, 2, NT).transpose(3, 1, 0, 2)
              .reshape(NT, D).astype(np.float32))
        out[c * NT:(c + 1) * NT, :] = yc
    out += bias[None, :]
    return out
